# revision 1
# baseline (speedup 1.0000x reference)
"""Trainium2 Bass kernel for nn_ConnectLoss (pairwise BCE-Dice instance loss).

Strategy (8 NeuronCores, pixel-sharded):
  - Each core gets H/8 = 256 rows (524288 pixels) of all four inputs.
  - Heavy part is the joint histogram inter[N=16, K=32] between target/pred
    instance labels. Per core: build fp16 one-hot indicator tiles on DVE
    (tensor_scalar is_equal, 4x mode) and contract 128 pixels/instruction on
    the TensorE into a PSUM-accumulated [16, 32+3] result. The 3 extra moving
    columns carry (cls, ln(cls), ln(1-cls)) so the same matmul also yields
    per-target-class sums needed for the cls_out BCE term.
  - sum(pred_score^2) via ACT Square with accum_out.
  - Marginals sp/st derive from inter row/col sums; tiny final math on host.

cls_out is uniform in [1e-4, 1-1e-4] so the torch-style -100 log clamp can
never trigger; logs are computed unclamped.
"""

import sys

if "/opt/trn_rl_repo" not in sys.path:
    sys.path.insert(0, "/opt/trn_rl_repo")

import numpy as np
from contextlib import ExitStack

# ---------------------------------------------------------------- constants
P = 128
H, W = 2048, 2048
NCORES = 8
ROWS = H // NCORES                 # 256 rows per core
PIX = ROWS * W                     # 524288 pixels per core
FPP = PIX // P                     # 4096 free elems per partition
CF = 1024                          # max chunk free size
CHUNKS = [1024, 1024, 1024, 768, 256]   # tapered tail: PE trails less
assert sum(CHUNKS) == FPP
NCHUNK = len(CHUNKS)
SCF = 512                          # max subchunk free size (target one-hots)
K = 32                             # pred instance classes
KB = K - 1                         # pred one-hot classes actually built
N = 16                             # target instance classes
NV = 4                             # extra cols: ones, cls, ln(cls), ln(1-cls)
MCOL = KB + NV                     # 35 moving columns
OUTC = 80                          # output cols: [0:35] inter+aux, [40:72] ps2

SMOOTH = 1.0
HWPIX = float(H * W)

_cached = {}
TRACE = False


def _build_bass():
    import concourse.bass as bass
    import concourse.bacc as bacc
    import concourse.mybir as mybir
    from concourse.tile import TileContext

    f32 = mybir.dt.float32
    f16 = mybir.dt.float16
    i32 = mybir.dt.int32
    u8 = mybir.dt.uint8
    eq = mybir.AluOpType.is_equal
    AF = mybir.ActivationFunctionType

    nc = bacc.Bacc("TRN2", num_swdge_queues=4)
    pm_d = nc.dram_tensor("pm", [PIX], i32, kind="ExternalInput")
    tm_d = nc.dram_tensor("tm", [PIX], i32, kind="ExternalInput")
    cls_d = nc.dram_tensor("cls", [PIX], f32, kind="ExternalInput")
    ps_d = nc.dram_tensor("ps", [PIX], f32, kind="ExternalInput")
    out_d = nc.dram_tensor("out", [P, OUTC], f32, kind="ExternalOutput")

    pm_v = pm_d[:].rearrange("(p f) -> p f", p=P)
    tm_v = tm_d[:].rearrange("(p f) -> p f", p=P)
    cls_v = cls_d[:].rearrange("(p f) -> p f", p=P)
    ps_v = ps_d[:].rearrange("(p f) -> p f", p=P)

    with ExitStack() as es:
        tc = es.enter_context(TileContext(nc))
        pool_in = es.enter_context(tc.tile_pool(name="inp", bufs=2))
        pool_toh = es.enter_context(tc.tile_pool(name="toh", bufs=2))
        pool_poh = es.enter_context(tc.tile_pool(name="poh", bufs=2))
        pool_misc = es.enter_context(tc.tile_pool(name="misc", bufs=1))
        pool_scr = es.enter_context(tc.tile_pool(name="scr", bufs=2))
        psum = es.enter_context(tc.tile_pool(name="ps", bufs=1, space="PSUM"))

        inter_ps = psum.tile([N, MCOL], f32)
        ps2acc = pool_misc.tile([P, NCHUNK], f32)

        off = 0
        for c, CFc in enumerate(CHUNKS):
            cs = slice(off, off + CFc)
            # labels arrive as fp16 via casting DMA (gpsimd SWDGE)
            pm16 = pool_in.tile([P, CFc], f16, tag="pm16")
            tm16 = pool_in.tile([P, CFc], f16, tag="tm16")
            cls_t = pool_in.tile([P, CFc], f32, tag="cls")
            ps_t = pool_in.tile([P, CFc], f16, tag="ps")
            nc.gpsimd.dma_start(out=pm16[:], in_=pm_v[:, cs])
            nc.gpsimd.dma_start(out=tm16[:], in_=tm_v[:, cs])
            nc.sync.dma_start(out=cls_t[:], in_=cls_v[:, cs])
            nc.gpsimd.dma_start(out=ps_t[:], in_=ps_v[:, cs])

            # pred one-hots + V columns at chunk granularity, class-major
            pohv = pool_poh.tile([P, MCOL * CFc], f16, tag="pohv")
            for k in range(KB):
                nc.vector.tensor_scalar(
                    pohv[:, k * CFc:(k + 1) * CFc], pm16[:], float(k), None, eq
                )
            # V columns on ACT: ones, cls, ln(cls), ln(1-cls)
            nc.scalar.activation(
                pohv[:, KB * CFc:(KB + 1) * CFc], cls_t[:], AF.Copy,
                bias=1.0, scale=0.0,
            )
            nc.scalar.activation(
                pohv[:, (KB + 1) * CFc:(KB + 2) * CFc], cls_t[:], AF.Copy
            )
            nc.scalar.activation(
                pohv[:, (KB + 2) * CFc:(KB + 3) * CFc], cls_t[:], AF.Ln
            )
            nc.scalar.activation(
                pohv[:, (KB + 3) * CFc:(KB + 4) * CFc], cls_t[:], AF.Ln,
                bias=1.0, scale=-1.0,
            )
            # sum(pred_score^2) per partition for this chunk
            scr = pool_scr.tile([P, CFc], f16, tag="scr")
            nc.scalar.activation(
                scr[:], ps_t[:], AF.Square,
                accum_out=ps2acc[:, c:c + 1],
            )

            poh3 = pohv[:].rearrange("p (m f) -> p f m", m=MCOL)
            soff = 0
            while soff < CFc:
                SCFc = min(SCF, CFc - soff)
                # target one-hots at subchunk granularity, class-major
                toh = pool_toh.tile([P, N * SCFc], f16, tag="toh")
                for n in range(N):
                    nc.vector.tensor_scalar(
                        toh[:, n * SCFc:(n + 1) * SCFc],
                        tm16[:, soff:soff + SCFc], float(n), None, eq,
                    )
                toh3 = toh[:].rearrange("p (n f) -> p f n", n=N)
                for jj in range(SCFc):
                    j = soff + jj
                    first = (c == 0 and j == 0)
                    last = (c == NCHUNK - 1 and j == CFc - 1)
                    nc.tensor.matmul(
                        inter_ps[:, :],
                        toh3[:, jj:jj + 1, :],
                        poh3[:, j:j + 1, :],
                        start=first,
                        stop=last,
                    )
                soff += SCFc
            off += CFc

        inter_sb = pool_misc.tile([N, MCOL], f32)
        nc.scalar.copy(inter_sb[:], inter_ps[:])
        nc.scalar.dma_start(out=out_d[0:N, 0:MCOL], in_=inter_sb[:])
        nc.scalar.dma_start(out=out_d[:, 40:40 + NCHUNK], in_=ps2acc[:])

    nc.finalize()
    return nc


def _get_nc():
    if "nc" not in _cached:
        _cached["nc"] = _build_bass()
    return _cached["nc"]


def _get_runner():
    """Build the sharded jitted executable ONCE; reuse across calls.

    Mirrors concourse.bass2jax.run_bass_via_pjrt's multi-core path, but caches
    the jitted function so repeat calls skip retrace/recompile.
    """
    if "runner" in _cached:
        return _cached["runner"]

    import jax
    import concourse.mybir as mybir
    from jax.sharding import Mesh, PartitionSpec
    from jax.experimental.shard_map import shard_map
    from concourse import bass2jax

    bass2jax.install_neuronx_cc_hook()
    nc = _get_nc()
    partition_name = (
        nc.partition_id_tensor.name if nc.partition_id_tensor else None
    )

    in_names, out_names, out_avals, zero_outs = [], [], [], []
    for alloc in nc.m.functions[0].allocations:
        if not isinstance(alloc, mybir.MemoryLocationSet):
            continue
        name = alloc.memorylocations[0].name
        if alloc.kind == "ExternalInput":
            if name != partition_name:
                in_names.append(name)
        elif alloc.kind == "ExternalOutput":
            out_names.append(name)
            shape = tuple(alloc.tensor_shape)
            dtype = mybir.dt.np(alloc.dtype)
            out_avals.append(jax.core.ShapedArray(shape, dtype))
            zero_outs.append(np.zeros(shape, dtype))
    n_params = len(in_names)
    n_outs = len(out_avals)
    all_in_names = list(in_names) + list(out_names)
    if partition_name is not None:
        all_in_names.append(partition_name)
    donate = tuple(range(n_params, n_params + n_outs))

    def _body(*args):
        operands = list(args)
        if partition_name is not None:
            operands.append(bass2jax.partition_id_tensor())
        outs = bass2jax._bass_exec_p.bind(
            *operands,
            out_avals=tuple(out_avals),
            in_names=tuple(all_in_names),
            out_names=tuple(out_names),
            lowering_input_output_aliases=(),
            sim_require_finite=True,
            sim_require_nnan=True,
            nc=nc,
        )
        return tuple(outs)

    devices = jax.devices()[:NCORES]
    mesh = Mesh(np.asarray(devices), ("core",))
    in_specs = (PartitionSpec("core"),) * (n_params + n_outs)
    out_specs = (PartitionSpec("core"),) * n_outs
    sharded = jax.jit(
        shard_map(
            _body, mesh=mesh, in_specs=in_specs, out_specs=out_specs,
            check_rep=False,
        ),
        donate_argnums=donate,
        keep_unused=True,
    )

    def run(in_maps):
        concat_in = [
            np.concatenate([np.asarray(m[name]) for m in in_maps], axis=0)
            for name in in_names
        ]
        concat_zeros = [
            np.zeros((NCORES * z.shape[0], *z.shape[1:]), z.dtype)
            for z in zero_outs
        ]
        out_arrs = sharded(*concat_in, *concat_zeros)
        return [
            {
                name: np.asarray(out_arrs[i]).reshape(
                    NCORES, *out_avals[i].shape)[c]
                for i, name in enumerate(out_names)
            }
            for c in range(NCORES)
        ]

    def bench(in_maps, iters=20):
        """Time the sharded call with device-resident inputs."""
        import time
        from jax.sharding import NamedSharding

        concat_in = [
            np.concatenate([np.asarray(m[name]) for m in in_maps], axis=0)
            for name in in_names
        ]
        shard = NamedSharding(mesh, PartitionSpec("core"))
        dev_in = [jax.device_put(x, shard) for x in concat_in]
        zeros = [
            np.zeros((NCORES * z.shape[0], *z.shape[1:]), z.dtype)
            for z in zero_outs
        ]

        def call():
            zs = [jax.device_put(z, shard) for z in zeros]
            outs = sharded(*dev_in, *zs)
            for o in outs:
                o.block_until_ready()

        call()
        ts = []
        for _ in range(iters):
            t0 = time.perf_counter()
            call()
            ts.append(time.perf_counter() - t0)
        return min(ts), sum(ts) / len(ts)

    run.bench = bench
    _cached["runner"] = run
    return run


def kernel(pred_instance_mask, pred_score, cls_out, target_mask):
    run = _get_runner()

    in_maps = []
    for c in range(NCORES):
        rs = slice(c * ROWS, (c + 1) * ROWS)
        in_maps.append({
            "pm": np.ascontiguousarray(
                pred_instance_mask[rs]).reshape(-1).astype(np.int32),
            "tm": np.ascontiguousarray(
                target_mask[rs]).reshape(-1).astype(np.int32),
            "cls": np.ascontiguousarray(
                cls_out[rs]).reshape(-1).astype(np.float32),
            "ps": np.ascontiguousarray(
                pred_score[rs]).reshape(-1).astype(np.float32),
        })

    outs = [r["out"] for r in run(in_maps)]

    acc = np.zeros((N, MCOL), dtype=np.float64)
    ps2 = 0.0
    for o in outs:
        o = o.astype(np.float64)
        acc += o[0:N, 0:MCOL]
        ps2 += o[:, 40:40 + NCHUNK].sum()

    inter31 = acc[:, 0:KB]
    st = acc[:, KB]
    inter = np.concatenate(
        [inter31, (st - inter31.sum(axis=1))[:, None]], axis=1)
    aux = acc[:, KB + 1:KB + 4]

    return _host_finish(inter, aux, ps2)


def _host_finish(inter, aux, ps2):
    st = inter.sum(axis=1)            # [N] target marginals
    sp = inter.sum(axis=0)            # [K] pred marginals
    sum_t = HWPIX - st[0]             # count(target > 0)
    sum_p = aux[:, 0].sum()           # sum(cls_out)
    sum_logp = aux[:, 1].sum()
    inter_cls = sum_p - aux[0, 0]     # sum over target>0 of cls_out
    bce_sum = (sum_logp - aux[0, 1]) + aux[0, 2]

    mse = ps2 / HWPIX
    bce_cls = -bce_sum / HWPIX
    dice_cls = 1.0 - (2.0 * inter_cls + SMOOTH) / (sum_p + sum_t + SMOOTH)

    union = st[:, None] + sp[None, :]
    bce_pair = 100.0 * (union - 2.0 * inter) / HWPIX
    dice_pair = 1.0 - (2.0 * inter + SMOOTH) / (union + SMOOTH)
    pair = bce_pair + dice_pair
    res = mse + bce_cls + dice_cls + pair.min(axis=1).sum()
    return np.float32(res / float(N))



# revision 2
# speedup vs baseline: 28.9876x; 28.9876x over previous
"""Trainium2 Bass kernel for nn_ConnectLoss (pairwise BCE-Dice instance loss).

Strategy (8 NeuronCores, pixel-sharded, one fused matmul stream):
  - Each core gets H/8 = 256 rows (524288 pixels = [128, 4096]) of all inputs.
  - ONE PSUM-accumulated matmul per 128-pixel column computes everything:
      stationary lhsT [128, 35] = [poh k=0..30 | ones | cls | ln cls | ln(1-cls)]
      moving    rhs  [128, 16] = [toh n=0..14 | ones]
      out [35, 16] accumulates over all 4096 columns.
    Missing class 31 / class 15 and all marginals are recovered on host from
    the ones row/column.
  - One-hot building is split across three engines per chunk:
      DVE:  tensor_scalar is_equal f16 (4x mode) — poh k<20 and toh n<15
      Pool: gpsimd tensor_scalar is_equal — poh k=20..25
      ACT:  Square(x-k) then Relu(1-y) (exact for integer labels) — poh 26..30
    All ACT functions (square/relu/copy/ln) live in one act table set.
  - ACT also does cls->f16 copy, Ln(cls), Ln(1-cls), Square(ps)+accum_out.
  - Tiny [35,16] + [128,6] outputs per core; final min/sum math on host.

cls_out is uniform in [1e-4, 1-1e-4] so the torch-style -100 log clamp can
never trigger; logs are computed unclamped.
"""

import sys

if "/opt/trn_rl_repo" not in sys.path:
    sys.path.insert(0, "/opt/trn_rl_repo")

import numpy as np
from contextlib import ExitStack

# ---------------------------------------------------------------- constants
P = 128
H, W = 2048, 2048
NCORES = 8
ROWS = H // NCORES                 # 256 rows per core
PIX = ROWS * W                     # 524288 pixels per core
FPP = PIX // P                     # 4096 free elems per partition
CF = 896                           # allocated chunk free size
CHUNKS = [896, 896, 896, 896, 512]
assert sum(CHUNKS) == FPP
NCHUNK = len(CHUNKS)
K = 32                             # pred instance classes
N = 16                             # target instance classes
SM = 35                            # stationary rows: 31 poh, ones, cls, ln, ln1m
MV = 16                            # moving cols: 15 toh, ones
OUTC = 24                          # out cols: [0:16] matmul, [16:16+NCHUNK] ps2

# poh class ranges per engine (classes 30,31 and targets 14,15 are
# reconstructed from value-column first moments, never built as one-hots)
DVE_KS = range(0, 23)
POOL_KS = range(23, 28)
ACT_KS = range(28, 30)

SMOOTH = 1.0
HWPIX = float(H * W)

_cached = {}


def _build_bass(rep=1, dve_ks=DVE_KS, pool_ks=POOL_KS, act_ks=ACT_KS,
                chunks=CHUNKS, dve_ns=range(N - 2), pool_ns=(), act_ns=(),
                cf=None):
    """rep>1 wraps the whole computation in a hardware loop executing it
    `rep` times back-to-back (identical work each pass; outputs are from the
    final pass). Used only for slope-based timing."""
    import concourse.bass as bass
    import concourse.bacc as bacc
    import concourse.mybir as mybir
    from concourse.tile import TileContext

    f32 = mybir.dt.float32
    f16 = mybir.dt.float16
    i32 = mybir.dt.int32
    eq = mybir.AluOpType.is_equal
    AF = mybir.ActivationFunctionType

    nc = bacc.Bacc("TRN2", num_swdge_queues=4)
    pm_d = nc.dram_tensor("pm", [PIX], i32, kind="ExternalInput")
    tm_d = nc.dram_tensor("tm", [PIX], i32, kind="ExternalInput")
    cls_d = nc.dram_tensor("cls", [PIX], f32, kind="ExternalInput")
    ps_d = nc.dram_tensor("ps", [PIX], f32, kind="ExternalInput")
    out_d = nc.dram_tensor("out", [P, OUTC], f32, kind="ExternalOutput")

    pm_v = pm_d[:].rearrange("(p f) -> p f", p=P)
    tm_v = tm_d[:].rearrange("(p f) -> p f", p=P)
    cls_v = cls_d[:].rearrange("(p f) -> p f", p=P)
    ps_v = ps_d[:].rearrange("(p f) -> p f", p=P)

    CFa = cf if cf is not None else max(chunks)
    with ExitStack() as es:
        tc = es.enter_context(TileContext(nc))
        pool_in = es.enter_context(tc.tile_pool(name="inp", bufs=2))
        pool_stat = es.enter_context(tc.tile_pool(name="stat", bufs=2))
        pool_mov = es.enter_context(tc.tile_pool(name="mov", bufs=2))
        pool_scr = es.enter_context(tc.tile_pool(name="scr", bufs=2))
        pool_misc = es.enter_context(tc.tile_pool(name="misc", bufs=1))
        psum = es.enter_context(tc.tile_pool(name="ps", bufs=1, space="PSUM"))

        acc_ps = psum.tile([SM, MV], f32)
        ps2acc = pool_misc.tile([P, len(chunks)], f32)

        # per-class bias constants for the ACT one-hot trick + relu bias
        act_all = list(act_ks) + list(act_ns)
        nb = len(act_all) + 1
        actb = pool_misc.tile([P, max(nb, 1)], f32)
        bias_col = {}
        for i, k in enumerate(act_all):
            nc.gpsimd.memset(actb[:, i:i + 1], -float(k))
            bias_col[k] = i
        nc.gpsimd.memset(actb[:, nb - 1:nb], 1.0)

        loop_ctx = tc.For_i(0, rep) if rep > 1 else None
        if loop_ctx is not None:
            loop_ctx.__enter__()

        off = 0
        nchunk = len(chunks)
        for c, CFc in enumerate(chunks):
            cs = slice(off, off + CFc)
            cls32 = pool_in.tile([P, CFa], f32, tag="cls32")
            ps32 = pool_in.tile([P, CFa], f32, tag="ps32")
            stat = pool_stat.tile([P, SM * CFa], f16, tag="stat")
            mov = pool_mov.tile([P, MV * CFa], f16, tag="mov")
            tmp = pool_scr.tile([P, CFa], f16, tag="tmp")

            # labels land (cast to f16) directly in their value-column slots:
            # pm -> stat row 30, tm -> mov col 14. The one-hot passes read
            # them from there.
            pm16 = stat[:, 30 * CFa:30 * CFa + CFc]
            tm16 = mov[:, 14 * CFa:14 * CFa + CFc]
            nc.gpsimd.dma_start(out=pm16, in_=pm_v[:, cs])
            nc.gpsimd.dma_start(out=tm16, in_=tm_v[:, cs])
            nc.sync.dma_start(out=cls32[:, :CFc], in_=cls_v[:, cs])
            nc.sync.dma_start(out=ps32[:, :CFc], in_=ps_v[:, cs])

            # ones rows: written once per buffer (chunks 0 and 1 cover both)
            if c < 2:
                nc.gpsimd.memset(stat[:, 31 * CFa:32 * CFa], 1.0)
                nc.gpsimd.memset(mov[:, 15 * CFa:16 * CFa], 1.0)

            # pred/target one-hots on ACT: Square(x-k) then Relu(1-y)
            relu_b = actb[:, nb - 1:nb]
            for k in act_ks:
                nc.scalar.activation(
                    tmp[:, :CFc], pm16, AF.Square,
                    bias=actb[:, bias_col[k]:bias_col[k] + 1],
                )
                nc.scalar.activation(
                    stat[:, k * CFa:k * CFa + CFc], tmp[:, :CFc], AF.Relu,
                    bias=relu_b, scale=-1.0,
                )
            for n in act_ns:
                nc.scalar.activation(
                    tmp[:, :CFc], tm16, AF.Square,
                    bias=actb[:, bias_col[n]:bias_col[n] + 1],
                )
                nc.scalar.activation(
                    mov[:, n * CFa:n * CFa + CFc], tmp[:, :CFc], AF.Relu,
                    bias=relu_b, scale=-1.0,
                )
            # one-hots on Pool (gpsimd)
            for k in pool_ks:
                nc.gpsimd.tensor_scalar(
                    stat[:, k * CFa:k * CFa + CFc], pm16,
                    float(k), None, eq,
                )
            for n in pool_ns:
                nc.gpsimd.tensor_scalar(
                    mov[:, n * CFa:n * CFa + CFc], tm16,
                    float(n), None, eq,
                )
            # one-hots on DVE (f16 4x)
            for k in dve_ks:
                nc.vector.tensor_scalar(
                    stat[:, k * CFa:k * CFa + CFc], pm16,
                    float(k), None, eq,
                )
            for n in dve_ns:
                nc.vector.tensor_scalar(
                    mov[:, n * CFa:n * CFa + CFc], tm16,
                    float(n), None, eq,
                )
            # aux stationary rows on ACT
            nc.scalar.activation(
                stat[:, 32 * CFa:32 * CFa + CFc], cls32[:, :CFc], AF.Copy)
            nc.scalar.activation(
                stat[:, 33 * CFa:33 * CFa + CFc], cls32[:, :CFc], AF.Ln)
            nc.scalar.activation(
                stat[:, 34 * CFa:34 * CFa + CFc], cls32[:, :CFc], AF.Ln,
                bias=1.0, scale=-1.0,
            )
            # sum(ps^2) partial for this chunk (Square output is discarded)
            nc.scalar.activation(
                tmp[:, :CFc], ps32[:, :CFc], AF.Square,
                accum_out=ps2acc[:, c:c + 1],
            )

            stat3 = stat[:].rearrange("p (m f) -> p f m", m=SM)
            mov3 = mov[:].rearrange("p (n f) -> p f n", n=MV)
            for j in range(CFc):
                first = (c == 0 and j == 0)
                last = (c == nchunk - 1 and j == CFc - 1)
                nc.tensor.matmul(
                    acc_ps[:, :],
                    stat3[:, j:j + 1, :],
                    mov3[:, j:j + 1, :],
                    start=first,
                    stop=last,
                )
            off += CFc

        if loop_ctx is not None:
            loop_ctx.__exit__(None, None, None)

        evac = pool_misc.tile([SM, MV], f32)
        nc.scalar.copy(evac[:], acc_ps[:])
        nc.scalar.dma_start(out=out_d[0:SM, 0:MV], in_=evac[:])
        nc.scalar.dma_start(out=out_d[:, 16:16 + len(chunks)], in_=ps2acc[:])

    nc.finalize()
    return nc


def _get_nc(rep=1):
    key = f"nc{rep}"
    if key not in _cached:
        _cached[key] = _build_bass(rep)
    return _cached[key]


def _get_runner():
    """Build the sharded jitted executable ONCE; reuse across calls."""
    if "runner" in _cached:
        return _cached["runner"]

    import jax
    import concourse.mybir as mybir
    from jax.sharding import Mesh, PartitionSpec
    from jax.experimental.shard_map import shard_map
    from concourse import bass2jax

    bass2jax.install_neuronx_cc_hook()
    nc = _get_nc()
    partition_name = (
        nc.partition_id_tensor.name if nc.partition_id_tensor else None
    )

    in_names, out_names, out_avals, zero_outs = [], [], [], []
    for alloc in nc.m.functions[0].allocations:
        if not isinstance(alloc, mybir.MemoryLocationSet):
            continue
        name = alloc.memorylocations[0].name
        if alloc.kind == "ExternalInput":
            if name != partition_name:
                in_names.append(name)
        elif alloc.kind == "ExternalOutput":
            out_names.append(name)
            shape = tuple(alloc.tensor_shape)
            dtype = mybir.dt.np(alloc.dtype)
            out_avals.append(jax.core.ShapedArray(shape, dtype))
            zero_outs.append(np.zeros(shape, dtype))
    n_params = len(in_names)
    n_outs = len(out_avals)
    all_in_names = list(in_names) + list(out_names)
    if partition_name is not None:
        all_in_names.append(partition_name)
    donate = tuple(range(n_params, n_params + n_outs))

    def _body(*args):
        operands = list(args)
        if partition_name is not None:
            operands.append(bass2jax.partition_id_tensor())
        outs = bass2jax._bass_exec_p.bind(
            *operands,
            out_avals=tuple(out_avals),
            in_names=tuple(all_in_names),
            out_names=tuple(out_names),
            lowering_input_output_aliases=(),
            sim_require_finite=True,
            sim_require_nnan=True,
            nc=nc,
        )
        return tuple(outs)

    devices = jax.devices()[:NCORES]
    mesh = Mesh(np.asarray(devices), ("core",))
    in_specs = (PartitionSpec("core"),) * (n_params + n_outs)
    out_specs = (PartitionSpec("core"),) * n_outs
    sharded = jax.jit(
        shard_map(
            _body, mesh=mesh, in_specs=in_specs, out_specs=out_specs,
            check_rep=False,
        ),
        donate_argnums=donate,
        keep_unused=True,
    )

    def run(in_maps):
        concat_in = [
            np.concatenate([np.asarray(m[name]) for m in in_maps], axis=0)
            for name in in_names
        ]
        concat_zeros = [
            np.zeros((NCORES * z.shape[0], *z.shape[1:]), z.dtype)
            for z in zero_outs
        ]
        out_arrs = sharded(*concat_in, *concat_zeros)
        return [
            {
                name: np.asarray(out_arrs[i]).reshape(
                    NCORES, *out_avals[i].shape)[c]
                for i, name in enumerate(out_names)
            }
            for c in range(NCORES)
        ]

    run.in_names = in_names
    _cached["runner"] = run
    return run


def _shard_inputs(pred_instance_mask, pred_score, cls_out, target_mask):
    in_maps = []
    for c in range(NCORES):
        rs = slice(c * ROWS, (c + 1) * ROWS)
        in_maps.append({
            "pm": np.ascontiguousarray(
                pred_instance_mask[rs]).reshape(-1).astype(np.int32),
            "tm": np.ascontiguousarray(
                target_mask[rs]).reshape(-1).astype(np.int32),
            "cls": np.ascontiguousarray(
                cls_out[rs]).reshape(-1).astype(np.float32),
            "ps": np.ascontiguousarray(
                pred_score[rs]).reshape(-1).astype(np.float32),
        })
    return in_maps


def kernel(pred_instance_mask, pred_score, cls_out, target_mask):
    run = _get_runner()
    in_maps = _shard_inputs(pred_instance_mask, pred_score, cls_out, target_mask)
    outs = [r["out"] for r in run(in_maps)]
    return _host_finish(outs)


def _host_finish(outs):
    O = np.zeros((SM, MV), dtype=np.float64)
    ps2 = 0.0
    for o in outs:
        o = o.astype(np.float64)
        O += o[0:SM, 0:MV]
        ps2 += o[:, 16:16 + NCHUNK].sum()

    # Layout: stat rows 0..29 poh | 30 pm | 31 ones | 32 cls | 33 ln | 34 ln1m
    #         mov cols 0..13 toh | 14 tm | 15 ones
    ks = np.arange(30, dtype=np.float64)
    ns = np.arange(14, dtype=np.float64)

    inter = np.zeros((N, K), dtype=np.float64)
    inter[0:14, 0:30] = O[0:30, 0:14].T
    Tm1 = O[0:30, 14]          # sum tm * poh_k
    sp30 = O[0:30, 15]         # sum poh_k
    Pm1 = O[30, 0:14]          # sum pm * toh_n
    X = O[30, 14]              # sum pm*tm (f32-rounded)
    P1 = O[30, 15]             # sum pm
    st14 = O[31, 0:14]         # st[n], n<14
    T1 = O[31, 14]             # sum tm
    PIXT = O[31, 15]           # total pixels

    # split target classes 14,15 for each pred k<30
    cc = sp30 - inter[0:14, 0:30].sum(axis=0)
    dd = Tm1 - (ns[:, None] * inter[0:14, 0:30]).sum(axis=0)
    inter[15, 0:30] = dd - 14.0 * cc
    inter[14, 0:30] = 15.0 * cc - dd

    # split pred classes 30,31 for each target n<14
    aa = st14 - inter[0:14, 0:30].sum(axis=1)
    bb = Pm1 - (inter[0:14, 0:30] * ks[None, :]).sum(axis=1)
    inter[0:14, 31] = bb - 30.0 * aa
    inter[0:14, 30] = 31.0 * aa - bb

    # marginals for the missing rows/cols
    A = PIXT - st14.sum()
    B = T1 - (ns * st14).sum()
    st15 = B - 14.0 * A
    st = np.concatenate([st14, [15.0 * A - B, st15]])
    st[14] = 15.0 * A - B
    sp_all = inter.sum(axis=0)
    Cc = PIXT - sp30.sum()
    Dd = P1 - (ks * sp30).sum()
    sp31 = Dd - 30.0 * Cc
    sp30f = 31.0 * Cc - Dd

    # 2x2 corner from row/col sums + the pm*tm cross moment
    E = st[14] - inter[14, 0:30].sum()
    F = st[15] - inter[15, 0:30].sum()
    G = sp30f - inter[0:14, 30].sum()
    Xres = X - (np.outer(ns, ks) * inter[0:14, 0:30]).sum() \
        - (14.0 * inter[14, 0:30] * ks).sum() \
        - (15.0 * inter[15, 0:30] * ks).sum() \
        - (ns * 30.0 * inter[0:14, 30]).sum() \
        - (ns * 31.0 * inter[0:14, 31]).sum()
    # 420x + 434(E-x) + 450(G-x) + 465(F-G+x) = Xres, coeff of x is +1
    i1430 = Xres - 434.0 * E - 450.0 * G - 465.0 * (F - G)
    inter[14, 30] = i1430
    inter[14, 31] = E - i1430
    inter[15, 30] = G - i1430
    inter[15, 31] = F - G + i1430

    sum_cls = O[32, 15]
    inter_cls = sum_cls - O[32, 0]
    bce_sum = (O[33, 15] - O[33, 0]) + O[34, 0]
    sum_t = HWPIX - st[0]

    mse = ps2 / HWPIX
    bce_cls = -bce_sum / HWPIX
    dice_cls = 1.0 - (2.0 * inter_cls + SMOOTH) / (sum_cls + sum_t + SMOOTH)

    sp = inter.sum(axis=0)
    union = st[:, None] + sp[None, :]
    bce_pair = 100.0 * (union - 2.0 * inter) / HWPIX
    dice_pair = 1.0 - (2.0 * inter + SMOOTH) / (union + SMOOTH)
    pair = bce_pair + dice_pair
    res = mse + bce_cls + dice_cls + pair.min(axis=1).sum()
    return np.float32(res / float(N))
